# revision 11
# baseline (speedup 1.0000x reference)
"""DLinear layer (nn_DLinearLayer) TRN2 Bass kernel.

Math (reference):
    trend[b,t,f]  = avgpool2(x)[b,t,f] = 0.5*(x[t]+x[t+1]), last: x[T-1]
    resid         = x - trend
    out[b,n,f]    = trend[:,:,f] @ trend_W[f] + trend_b[f,n]
                  + resid[:,:,f] @ residual_W[f] + residual_b[f,n]

Both trend and resid are fixed linear maps of x along t (trend = M x with
M bidiagonal, resid = (I-M) x), so the whole layer folds into ONE GEMM:

    out[:, :, f] = x[:, :, f] @ Wc[f] + (tb+rb)[f]
    Wc[f] = residual_W[f] + M^T (trend_W[f] - residual_W[f])
    (M^T D)[t] = 0.5*(D[t] + D[t-1]),  edges: t=0 -> 0.5*D[0],
                 t=T-1 -> D[T-1] + 0.5*D[T-2]

The fold runs on host (weights are read once anyway), halving both PE
work and weight DMA vs the two-GEMM formulation. The bias row is added
on host after the gather (it is all-zeros in this model). On-device
dtype is fp16 (1 PE cycle/row, half the HBM bytes of fp32r; e5m10 keeps
rel-l2 ~4e-4 at K=1024, far under the 2e-2 gate).

Sharding: feature-expert — core k owns features {2k, 2k+1}; each
feature's [B,T] x [T,N] GEMM is independent and every weight byte is
moved exactly once across the system.

Schedule (trace-derived):
  * Two HWDGE rings (SP + ACT) stream x and W in consumption order,
    ~equal bytes per ring (sustains ~400 GB/s). The first x chunk and
    first half-W-chunk are split small so the first matmul's inputs
    land ~2us earlier. SWDGE is NOT used on the critical path (its
    desc-gen + sem path costs ~4us); it only carries early-feature
    output stores, which are latency-tolerant.
  * The PE p-state ramp (~1.2 GHz for the first ~3us of busy time,
    2.4 GHz after) is absorbed by warm-up matmuls on zeroed SBUF timed
    to end right as the first real inputs arrive.
  * Matmuls are h-major per chunk so a half-chunk is consumable on
    arrival; the final W chunk is h-split across both rings.
  * Tail drains: copies alternate ACT/DVE per psum, stores are packed
    so ring issues overlap the remaining copies.
"""

import numpy as np

import concourse.bass as bass
import concourse.mybir as mybir
import concourse.tile as tile
from concourse.bass_utils import run_bass_kernel_spmd

F, B, T, N = 16, 256, 1024, 1024
NCORES = 8
FL = F // NCORES          # features per core
TC = T // 128             # t chunks (contraction tiles)
NB = B // 128             # batch tiles (output partition tiles)
NH = N // 512             # output free-dim halves
HC = TC // 2
F32 = mybir.dt.float32
F16 = mybir.dt.float16
NWARM = 5                 # PE ramp warm-up matmuls


def _split_multi_waits(nc):
    """This container's walrus build accepts at most ONE sem wait per
    instruction ("Too many sync wait commands" in CoreV3Gen setupSyncWait).
    Tile emits 2+. Move excess waits onto nofuse NoOps placed immediately
    before the owning instruction on the same engine: engines execute their
    stream in order, so semantics are unchanged."""
    for fn in nc.m.functions:
        for blk in fn.blocks:
            out = []
            for inst in blk.instructions:
                si = inst.sync_info
                if si is not None and si.on_wait and len(si.on_wait) > 1:
                    waits = list(si.on_wait)
                    for j, w in enumerate(waits[:-1]):
                        out.append(mybir.InstNoOp(
                            name=f"{inst.name}-ws{j}",
                            engine=inst.engine,
                            bass_nofuse=True,
                            sync_info=mybir.SyncInfo(on_wait=[w], on_update=[]),
                        ))
                    si.on_wait = [waits[-1]]
                out.append(inst)
            blk.instructions[:] = out


def _build():
    nc = bass.Bass(trn_type="TRN2")

    x_d = nc.dram_tensor("x", [FL, 128, TC, B], F16, kind="ExternalInput")
    wc_d = nc.dram_tensor("Wc", [FL, T, N], F16, kind="ExternalInput")
    out_d = nc.dram_tensor("out", [FL, B, N], F16, kind="ExternalOutput")

    with tile.TileContext(nc) as tc:
        with (
            tc.tile_pool(name="wp", bufs=FL * TC) as wp,
            tc.tile_pool(name="xp", bufs=FL) as xp,
            tc.tile_pool(name="obuf", bufs=FL * NB * NH) as obp,
            tc.tile_pool(name="warm", bufs=1) as wmp,
            tc.tile_pool(name="ps", bufs=8, space="PSUM") as psp,
        ):
            hwdge = [nc.sync, nc.scalar]

            # PE warm-up source: a K=1 strip — the matmul still streams
            # 512 moving rows (same PE busy time as a full tile) but the
            # prerequisite memset is ~50ns, so warm-up starts right at
            # the ~7.3us body barrier.
            wtile = wmp.tile([1, 640], F16, tag="wm", name="warm")
            nc.vector.memset(wtile[:], 0.0)
            ps_warm = psp.tile([128, 512], F32, tag="ps", name="ps_warm")

            xs, wc = {}, {}
            for f in range(FL):
                xs[f] = xp.tile([128, TC, B], F16, tag="x", name=f"x_{f}")
                for c in range(TC):
                    wc[f, c] = wp.tile([128, N], F16, tag="w",
                                       name=f"w_{f}_{c}")

            # ---- DMA choreography (v2 layout + small first transfers).
            # f0: SP: x0 (c0 split small), then W c2/c4/c6;
            #     ACT: W00 (h-split), W01, W03, W05, W07.
            nc.sync.dma_start(xs[0][:, 0:1, :], x_d[0, :, 0:1, :])
            nc.scalar.dma_start(wc[0, 0][:, 0:512], wc_d[0, 0:128, 0:512])
            nc.sync.dma_start(xs[0][:, 1:HC, :], x_d[0, :, 1:HC, :])
            nc.scalar.dma_start(wc[0, 0][:, 512:N], wc_d[0, 0:128, 512:N])
            nc.sync.dma_start(wc[0, 2][:], wc_d[0, 2 * 128:3 * 128, :])
            nc.sync.dma_start(xs[0][:, HC:TC, :], x_d[0, :, HC:TC, :])
            for c in range(1, TC):
                if c == 2:
                    continue
                ring = nc.scalar if c % 2 == 1 else nc.sync
                ring.dma_start(wc[0, c][:], wc_d[0, c * 128:(c + 1) * 128, :])

            # f1 (mirrored): ACT: x1 halves, W c2/c4/c6;
            #                SP: W c0, c1, c3, c5; last chunk h-split.
            nc.scalar.dma_start(xs[1][:, 0:HC, :], x_d[1, :, 0:HC, :])
            nc.sync.dma_start(wc[1, 0][:], wc_d[1, 0:128, :])
            nc.scalar.dma_start(xs[1][:, HC:TC, :], x_d[1, :, HC:TC, :])
            for c in range(1, TC - 1):
                ring = nc.sync if c % 2 == 1 else nc.scalar
                ring.dma_start(wc[1, c][:], wc_d[1, c * 128:(c + 1) * 128, :])
            cl = TC - 1
            nc.sync.dma_start(wc[1, cl][:, 0:512],
                              wc_d[1, cl * 128:(cl + 1) * 128, 0:512])
            nc.scalar.dma_start(wc[1, cl][:, 512:N],
                                wc_d[1, cl * 128:(cl + 1) * 128, 512:N])

            # ---- PE: warm-up absorbs the p-state ramp during DMA
            # spin-up, then the real stream (h-major per chunk); psum
            # (b,h) accumulates c=0..TC-1.
            for i in range(NWARM):
                nc.tensor.matmul(ps_warm[:], wtile[:, 0:128],
                                 wtile[:, 128:640], start=True, stop=True)

            for f in range(FL):
                psums = {(b, h): psp.tile([128, 512], F32, tag="ps",
                                          name=f"ps_{f}_{b}_{h}")
                         for b in range(NB) for h in range(NH)}
                for c in range(TC):
                    for h in range(NH):
                        ns = slice(h * 512, (h + 1) * 512)
                        for b in range(NB):
                            nc.tensor.matmul(
                                psums[b, h][:],
                                xs[f][:, c, b * 128:(b + 1) * 128],
                                wc[f, c][:, ns],
                                start=(c == 0), stop=(c == TC - 1))
                tail = f == FL - 1
                ots = {}
                for b in range(NB):
                    for h in range(NH):
                        ots[b, h] = obp.tile([128, 512], F16, tag="o",
                                             name=f"o_{f}_{b}_{h}")
                if not tail:
                    # early feature: all copies on DVE; stores enqueue on
                    # the HWDGE rings BEHIND every W chunk (ring FIFO), so
                    # they can never delay the weight stream.
                    for h in range(NH):
                        for b in range(NB):
                            nc.vector.tensor_scalar_mul(
                                ots[b, h][:], psums[b, h][:], 1.0)
                    for h in range(NH):
                        for b in range(NB):
                            bs = slice(b * 128, (b + 1) * 128)
                            ns = slice(h * 512, (h + 1) * 512)
                            hwdge[b].dma_start(out_d[f, bs, ns],
                                               ots[b, h][:])
                else:
                    # tail: copies split ACT/DVE by b; stores: SP takes
                    # the b0 stripes, ACT the b1 stripes after its copies.
                    nc.scalar.copy(ots[0, 0][:], psums[0, 0][:])
                    nc.vector.tensor_scalar_mul(
                        ots[1, 0][:], psums[1, 0][:], 1.0)
                    nc.scalar.copy(ots[0, 1][:], psums[0, 1][:])
                    nc.vector.tensor_scalar_mul(
                        ots[1, 1][:], psums[1, 1][:], 1.0)
                    nc.sync.dma_start(out_d[f, 0:128, 0:512], ots[0, 0][:])
                    nc.scalar.dma_start(out_d[f, 128:256, 0:512],
                                        ots[1, 0][:])
                    nc.sync.dma_start(out_d[f, 0:128, 512:N], ots[0, 1][:])
                    nc.scalar.dma_start(out_d[f, 128:256, 512:N],
                                        ots[1, 1][:])

    _split_multi_waits(nc)
    return nc


_NC_CACHE = []


def kernel(**inputs) -> np.ndarray:
    x = np.asarray(inputs["history_in"], dtype=np.float32)     # [B, T, F]
    wt = np.asarray(inputs["trend_W"], dtype=np.float32)       # [F, T, N]
    wr = np.asarray(inputs["residual_W"], dtype=np.float32)    # [F, T, N]
    tb = np.asarray(inputs["trend_b"], dtype=np.float32)       # [F, N]
    rb = np.asarray(inputs["residual_b"], dtype=np.float32)    # [F, N]

    # fold avgpool into the weights: Wc = Wr + M^T (Wt - Wr)
    d = wt - wr
    md = np.empty_like(d)
    md[:, 0] = 0.5 * d[:, 0]
    md[:, 1:T - 1] = 0.5 * (d[:, 1:T - 1] + d[:, 0:T - 2])
    md[:, T - 1] = d[:, T - 1] + 0.5 * d[:, T - 2]
    wcomb = (wr + md).astype(np.float16)                       # [F, T, N]

    xT = x.transpose(2, 1, 0)                                  # [F, T, B] view
    # partition-major: xpm[f, p, c, b] = xT[f, c*128+p, b]
    xpm = np.ascontiguousarray(
        xT.reshape(F, TC, 128, B).transpose(0, 2, 1, 3)).astype(np.float16)

    if not _NC_CACHE:
        _NC_CACHE.append(_build())
    nc = _NC_CACHE[0]

    in_maps = []
    for k in range(NCORES):
        sl = slice(FL * k, FL * (k + 1))
        in_maps.append({
            "x": np.ascontiguousarray(xpm[sl]),
            "Wc": np.ascontiguousarray(wcomb[sl]),
        })

    res = run_bass_kernel_spmd(nc, in_maps, core_ids=list(range(NCORES)))
    full = np.concatenate([r["out"] for r in res.results], axis=0)  # [F, B, N]
    out = full.astype(np.float32).transpose(1, 2, 0)                # [B, N, F]
    out = out + (tb + rb).T[None]
    return np.ascontiguousarray(out)


# revision 12
# speedup vs baseline: 1.0769x; 1.0769x over previous
"""DLinear layer (nn_DLinearLayer) TRN2 Bass kernel.

Math (reference):
    trend[b,t,f]  = avgpool2(x)[b,t,f] = 0.5*(x[t]+x[t+1]), last: x[T-1]
    resid         = x - trend
    out[b,n,f]    = trend[:,:,f] @ trend_W[f] + trend_b[f,n]
                  + resid[:,:,f] @ residual_W[f] + residual_b[f,n]

Both trend and resid are fixed linear maps of x along t (trend = M x with
M bidiagonal, resid = (I-M) x), so the whole layer folds into ONE GEMM:

    out[:, :, f] = x[:, :, f] @ Wc[f] + (tb+rb)[f]
    Wc[f] = residual_W[f] + M^T (trend_W[f] - residual_W[f])
    (M^T D)[t] = 0.5*(D[t] + D[t-1]),  edges: t=0 -> 0.5*D[0],
                 t=T-1 -> D[T-1] + 0.5*D[T-2]

The fold runs on host (weights are read once anyway), halving both PE
work and weight DMA vs the two-GEMM formulation. The bias row is added
on host after the gather (it is all-zeros in this model). On-device
dtype is fp16 (1 PE cycle/row, half the HBM bytes of fp32r; e5m10 keeps
rel-l2 ~4e-4 at K=1024, far under the 2e-2 gate).

Sharding: feature-expert — core k owns features {2k, 2k+1}; each
feature's [B,T] x [T,N] GEMM is independent and every weight byte is
moved exactly once across the system.

Schedule (trace-derived):
  * Kernel body can't start before the ~7.3us framework barrier; HWDGE
    ring feed is ~0.7us per dma_start and early per-DMA latency is
    1.5-2.5us, so the head uses small transfers (first matmul inputs
    land early) and the middle uses 2-chunk 512KB slabs (both x and W
    are partition-major on host, so any chunk range is one DMA with
    2-4KB descriptors).
  * The PE p-state ramp (~1.2 GHz until ~3us of sustained busy) is
    absorbed by K=1 warm-up matmuls (512 moving rows each, input is a
    64-byte memset strip) timed to end as the first real inputs land.
  * Matmuls are h-major per chunk; psum (b,h) accumulates c=0..TC-1.
    The final W chunk is h-split across both rings to shorten the tail.
  * Early-feature drains: copies on DVE, stores via SWDGE (gpsimd) so
    the W rings are never blocked. Tail drains alternate ACT/DVE copies
    and SP/ACT store rings.
"""

import numpy as np

import concourse.bass as bass
import concourse.mybir as mybir
import concourse.tile as tile
from concourse.bass_utils import run_bass_kernel_spmd

F, B, T, N = 16, 256, 1024, 1024
NCORES = 8
FL = F // NCORES          # features per core
TC = T // 128             # t chunks (contraction tiles)
NB = B // 128             # batch tiles (output partition tiles)
NH = N // 512             # output free-dim halves
HC = TC // 2
F32 = mybir.dt.float32
F16 = mybir.dt.float16
NWARM = 5                 # PE ramp warm-up matmuls


def _split_multi_waits(nc):
    """This container's walrus build accepts at most ONE sem wait per
    instruction ("Too many sync wait commands" in CoreV3Gen setupSyncWait).
    Tile emits 2+. Move excess waits onto nofuse NoOps placed immediately
    before the owning instruction on the same engine: engines execute their
    stream in order, so semantics are unchanged."""
    for fn in nc.m.functions:
        for blk in fn.blocks:
            out = []
            for inst in blk.instructions:
                si = inst.sync_info
                if si is not None and si.on_wait and len(si.on_wait) > 1:
                    waits = list(si.on_wait)
                    for j, w in enumerate(waits[:-1]):
                        out.append(mybir.InstNoOp(
                            name=f"{inst.name}-ws{j}",
                            engine=inst.engine,
                            bass_nofuse=True,
                            sync_info=mybir.SyncInfo(on_wait=[w], on_update=[]),
                        ))
                    si.on_wait = [waits[-1]]
                out.append(inst)
            blk.instructions[:] = out


def _build():
    nc = bass.Bass(trn_type="TRN2")

    x_d = nc.dram_tensor("x", [FL, 128, TC, B], F16, kind="ExternalInput")
    wc_d = nc.dram_tensor("Wc", [FL, 128, TC, N], F16, kind="ExternalInput")
    out_d = nc.dram_tensor("out", [FL, B, N], F16, kind="ExternalOutput")

    with tile.TileContext(nc) as tc:
        with (
            tc.tile_pool(name="wp", bufs=FL) as wp,
            tc.tile_pool(name="xp", bufs=FL) as xp,
            tc.tile_pool(name="obuf", bufs=FL * NB) as obp,
            tc.tile_pool(name="warm", bufs=1) as wmp,
            tc.tile_pool(name="ps", bufs=8, space="PSUM") as psp,
        ):
            # PE warm-up source: K=1 strip — each warm-up matmul still
            # streams 512 moving rows (real PE busy time) but the
            # prerequisite memset is ~50ns.
            wtile = wmp.tile([1, 640], F16, tag="wm", name="warm")
            nc.vector.memset(wtile[:], 0.0)
            ps_warm = psp.tile([128, 512], F32, tag="ps", name="ps_warm")

            xs, ws = {}, {}
            for f in range(FL):
                xs[f] = xp.tile([128, TC, B], F16, tag="x", name=f"x_{f}")
                ws[f] = wp.tile([128, TC, N], F16, tag="w", name=f"w_{f}")

            def wdma(ring, f, c0, c1, n0=0, n1=N):
                ring.dma_start(ws[f][:, c0:c1, n0:n1],
                               wc_d[f, :, c0:c1, n0:n1])

            # ---- DMA choreography: small head, 512KB slabs mid-stream,
            # h-split final chunk.
            nc.sync.dma_start(xs[0][:, 0:1, :], x_d[0, :, 0:1, :])     # 64K
            wdma(nc.scalar, 0, 0, 1, 0, 512)                           # 128K
            nc.sync.dma_start(xs[0][:, 1:HC, :], x_d[0, :, 1:HC, :])   # 192K
            wdma(nc.scalar, 0, 0, 1, 512, N)                           # 128K
            nc.sync.dma_start(xs[0][:, HC:TC, :], x_d[0, :, HC:TC, :])  # 256K
            wdma(nc.scalar, 0, 1, 2)                                   # 256K
            wdma(nc.sync, 0, 2, 4)                                     # 512K
            wdma(nc.scalar, 0, 4, 6)                                   # 512K
            wdma(nc.sync, 0, 6, 8)                                     # 512K
            nc.scalar.dma_start(xs[1][:, 0:HC, :], x_d[1, :, 0:HC, :])
            wdma(nc.sync, 1, 0, 2)                                     # 512K
            nc.scalar.dma_start(xs[1][:, HC:TC, :], x_d[1, :, HC:TC, :])
            wdma(nc.scalar, 1, 2, 4)                                   # 512K
            wdma(nc.sync, 1, 4, 6)                                     # 512K
            wdma(nc.scalar, 1, 6, 7)                                   # 256K
            wdma(nc.sync, 1, 7, 8, 0, 512)                             # 128K
            wdma(nc.scalar, 1, 7, 8, 512, N)                           # 128K

            # ---- PE: warm-up absorbs the p-state ramp during DMA
            # spin-up, then the real stream (h-major per chunk).
            for i in range(NWARM):
                nc.tensor.matmul(ps_warm[:], wtile[:, 0:128],
                                 wtile[:, 128:640], start=True, stop=True)

            for f in range(FL):
                psums = {(b, h): psp.tile([128, 512], F32, tag="ps",
                                          name=f"ps_{f}_{b}_{h}")
                         for b in range(NB) for h in range(NH)}
                for c in range(TC):
                    for h in range(NH):
                        ns = slice(h * 512, (h + 1) * 512)
                        for b in range(NB):
                            nc.tensor.matmul(
                                psums[b, h][:],
                                xs[f][:, c, b * 128:(b + 1) * 128],
                                ws[f][:, c, ns],
                                start=(c == 0), stop=(c == TC - 1))
                tail = f == FL - 1
                if not tail:
                    # early feature: copies on DVE into one [128, N] tile
                    # per b-stripe; 2x 256KB stores via SWDGE (gpsimd) so
                    # the W rings are never touched.
                    ots = {b: obp.tile([128, N], F16, tag="o",
                                       name=f"o_{f}_{b}")
                           for b in range(NB)}
                    for h in range(NH):
                        ns = slice(h * 512, (h + 1) * 512)
                        for b in range(NB):
                            nc.vector.tensor_scalar_mul(
                                ots[b][:, ns], psums[b, h][:], 1.0)
                    for b in range(NB):
                        bs = slice(b * 128, (b + 1) * 128)
                        nc.gpsimd.dma_start(out_d[f, bs, :], ots[b][:])
                else:
                    # tail: minimize last-matmul -> last-store. Copies:
                    # ACT takes b0 stripes, DVE takes b1; stores: SP takes
                    # b0? No — SP takes the two earliest-ready, ACT the
                    # rest after its copies.
                    ots = {(b, h): obp.tile([128, 512], F16, tag="ot",
                                            name=f"o_{f}_{b}_{h}")
                           for b in range(NB) for h in range(NH)}
                    nc.scalar.copy(ots[0, 0][:], psums[0, 0][:])
                    nc.vector.tensor_scalar_mul(
                        ots[1, 0][:], psums[1, 0][:], 1.0)
                    nc.scalar.copy(ots[0, 1][:], psums[0, 1][:])
                    nc.vector.tensor_scalar_mul(
                        ots[1, 1][:], psums[1, 1][:], 1.0)
                    nc.sync.dma_start(out_d[f, 0:128, 0:512], ots[0, 0][:])
                    nc.sync.dma_start(out_d[f, 128:256, 0:512], ots[1, 0][:])
                    nc.scalar.dma_start(out_d[f, 0:128, 512:N], ots[0, 1][:])
                    nc.scalar.dma_start(out_d[f, 128:256, 512:N],
                                        ots[1, 1][:])

    _split_multi_waits(nc)
    return nc


_NC_CACHE = []


def kernel(**inputs) -> np.ndarray:
    x = np.asarray(inputs["history_in"], dtype=np.float32)     # [B, T, F]
    wt = np.asarray(inputs["trend_W"], dtype=np.float32)       # [F, T, N]
    wr = np.asarray(inputs["residual_W"], dtype=np.float32)    # [F, T, N]
    tb = np.asarray(inputs["trend_b"], dtype=np.float32)       # [F, N]
    rb = np.asarray(inputs["residual_b"], dtype=np.float32)    # [F, N]

    # fold avgpool into the weights: Wc = Wr + M^T (Wt - Wr)
    d = wt - wr
    md = np.empty_like(d)
    md[:, 0] = 0.5 * d[:, 0]
    md[:, 1:T - 1] = 0.5 * (d[:, 1:T - 1] + d[:, 0:T - 2])
    md[:, T - 1] = d[:, T - 1] + 0.5 * d[:, T - 2]
    wcomb = (wr + md).astype(np.float16)                       # [F, T, N]
    # partition-major: wpm[f, p, c, n] = wcomb[f, c*128+p, n]
    wpm = np.ascontiguousarray(
        wcomb.reshape(F, TC, 128, N).transpose(0, 2, 1, 3))    # [F,128,TC,N]

    xT = x.transpose(2, 1, 0)                                  # [F, T, B] view
    # partition-major: xpm[f, p, c, b] = xT[f, c*128+p, b]
    xpm = np.ascontiguousarray(
        xT.reshape(F, TC, 128, B).transpose(0, 2, 1, 3)).astype(np.float16)

    if not _NC_CACHE:
        _NC_CACHE.append(_build())
    nc = _NC_CACHE[0]

    in_maps = []
    for k in range(NCORES):
        sl = slice(FL * k, FL * (k + 1))
        in_maps.append({
            "x": np.ascontiguousarray(xpm[sl]),
            "Wc": np.ascontiguousarray(wpm[sl]),
        })

    res = run_bass_kernel_spmd(nc, in_maps, core_ids=list(range(NCORES)))
    full = np.concatenate([r["out"] for r in res.results], axis=0)  # [F, B, N]
    out = full.astype(np.float32).transpose(1, 2, 0)                # [B, N, F]
    out = out + (tb + rb).T[None]
    return np.ascontiguousarray(out)


# revision 13
# speedup vs baseline: 1.1523x; 1.0700x over previous
"""v2 reference kernel (folded single-GEMM fp16) for variance calibration."""

import numpy as np

import concourse.bass as bass
import concourse.mybir as mybir
import concourse.tile as tile
from concourse.bass_utils import run_bass_kernel_spmd

F, B, T, N = 16, 256, 1024, 1024
NCORES = 8
FL = F // NCORES
TC = T // 128
NB = B // 128
NH = N // 512
HC = TC // 2
F32 = mybir.dt.float32
F16 = mybir.dt.float16


def _split_multi_waits(nc):
    for fn in nc.m.functions:
        for blk in fn.blocks:
            out = []
            for inst in blk.instructions:
                si = inst.sync_info
                if si is not None and si.on_wait and len(si.on_wait) > 1:
                    waits = list(si.on_wait)
                    for j, w in enumerate(waits[:-1]):
                        out.append(mybir.InstNoOp(
                            name=f"{inst.name}-ws{j}",
                            engine=inst.engine,
                            bass_nofuse=True,
                            sync_info=mybir.SyncInfo(on_wait=[w], on_update=[]),
                        ))
                    si.on_wait = [waits[-1]]
                out.append(inst)
            blk.instructions[:] = out


def _build():
    nc = bass.Bass(trn_type="TRN2")

    x_d = nc.dram_tensor("x", [FL, 128, TC, B], F16, kind="ExternalInput")
    wc_d = nc.dram_tensor("Wc", [FL, T, N], F16, kind="ExternalInput")
    out_d = nc.dram_tensor("out", [FL, B, N], F16, kind="ExternalOutput")

    with tile.TileContext(nc) as tc:
        with (
            tc.tile_pool(name="wp", bufs=2 * FL * TC) as wp,
            tc.tile_pool(name="xp", bufs=FL) as xp,
            tc.tile_pool(name="obuf", bufs=FL * NB * NH) as obp,
            tc.tile_pool(name="ps", bufs=8, space="PSUM") as psp,
        ):
            hwdge = [nc.sync, nc.scalar]

            xs, wc = {}, {}
            for f in range(FL):
                L = hwdge[f % 2]
                O = hwdge[1 - f % 2]
                xt = xp.tile([128, TC, B], F16, tag="x", name=f"x_{f}")
                xs[f] = xt
                L.dma_start(xt[:, 0:HC, :], x_d[f, :, 0:HC, :])
                wt0 = wp.tile([128, N], F16, tag="w", name=f"w_{f}_0")
                O.dma_start(wt0[:], wc_d[f, 0:128, :])
                wc[f, 0] = wt0
                L.dma_start(xt[:, HC:TC, :], x_d[f, :, HC:TC, :])
                for c in range(1, TC):
                    w = wp.tile([128, N], F16, tag="w", name=f"w_{f}_{c}")
                    if f == FL - 1 and c == TC - 1:
                        for h in range(NH):
                            ns = slice(h * 512, (h + 1) * 512)
                            hwdge[h % 2].dma_start(
                                w[:, ns], wc_d[f, c * 128:(c + 1) * 128, ns])
                    else:
                        ring = L if (c % 2 == 0) else O
                        ring.dma_start(w[:], wc_d[f, c * 128:(c + 1) * 128, :])
                    wc[f, c] = w

            for f in range(FL):
                psums = {(b, h): psp.tile([128, 512], F32, tag="ps",
                                          name=f"ps_{f}_{b}_{h}")
                         for b in range(NB) for h in range(NH)}
                for c in range(TC):
                    last = c == TC - 1
                    order = ([(h, b) for h in range(NH) for b in range(NB)]
                             if last else
                             [(h, b) for b in range(NB) for h in range(NH)])
                    for h, b in order:
                        ns = slice(h * 512, (h + 1) * 512)
                        nc.tensor.matmul(
                            psums[b, h][:],
                            xs[f][:, c, b * 128:(b + 1) * 128],
                            wc[f, c][:, ns],
                            start=(c == 0), stop=last)
                tail = f == FL - 1
                for h in range(NH):
                    for b in range(NB):
                        ns = slice(h * 512, (h + 1) * 512)
                        bs = slice(b * 128, (b + 1) * 128)
                        ot = obp.tile([128, 512], F16, tag="o",
                                      name=f"o_{f}_{b}_{h}")
                        if tail:
                            if b % 2 == 0:
                                nc.scalar.copy(ot[:], psums[b, h][:])
                            else:
                                nc.vector.tensor_scalar_mul(
                                    ot[:], psums[b, h][:], 1.0)
                            hwdge[b % 2].dma_start(out_d[f, bs, ns], ot[:])
                        else:
                            nc.vector.tensor_scalar_mul(
                                ot[:], psums[b, h][:], 1.0)
                            nc.gpsimd.dma_start(out_d[f, bs, ns], ot[:])

    _split_multi_waits(nc)
    return nc


_NC_CACHE = []


def kernel(**inputs) -> np.ndarray:
    x = np.asarray(inputs["history_in"], dtype=np.float32)
    wt = np.asarray(inputs["trend_W"], dtype=np.float32)
    wr = np.asarray(inputs["residual_W"], dtype=np.float32)
    tb = np.asarray(inputs["trend_b"], dtype=np.float32)
    rb = np.asarray(inputs["residual_b"], dtype=np.float32)

    d = wt - wr
    md = np.empty_like(d)
    md[:, 0] = 0.5 * d[:, 0]
    md[:, 1:T - 1] = 0.5 * (d[:, 1:T - 1] + d[:, 0:T - 2])
    md[:, T - 1] = d[:, T - 1] + 0.5 * d[:, T - 2]
    wcomb = (wr + md).astype(np.float16)

    xT = x.transpose(2, 1, 0)
    xpm = np.ascontiguousarray(
        xT.reshape(F, TC, 128, B).transpose(0, 2, 1, 3)).astype(np.float16)

    if not _NC_CACHE:
        _NC_CACHE.append(_build())
    nc = _NC_CACHE[0]

    in_maps = []
    for k in range(NCORES):
        sl = slice(FL * k, FL * (k + 1))
        in_maps.append({
            "x": np.ascontiguousarray(xpm[sl]),
            "Wc": np.ascontiguousarray(wcomb[sl]),
        })

    res = run_bass_kernel_spmd(nc, in_maps, core_ids=list(range(NCORES)))
    full = np.concatenate([r["out"] for r in res.results], axis=0)
    out = full.astype(np.float32).transpose(1, 2, 0)
    out = out + (tb + rb).T[None]
    return np.ascontiguousarray(out)
